# revision 49
# baseline (speedup 1.0000x reference)
"""Trainium2 Bass kernel for nn_MOTASG_KO_Reg (ragged graph-conv KO regression).

Strategy (8 NeuronCores, data-parallel over node rows):
  - N=16384 nodes = 16 batch samples x 1024 entities. Core c owns rows
    [2048c, 2048c+2048) = batch samples 2c, 2c+1.
  - Activations kept feature-major ("transposed", [feat, rows]) on chip so
    every linear is a native PE matmul (fp16 operands, fp32 PSUM).
  - name/desc path computed once on 128 entities/core, AllGathered, folded
    into cross via a fused vector add from an SBUF copy.
  - gconv1 segment-sum via dma_gather + one-hot scatter matmuls in PSUM.
    Edges whose dst never feeds gconv2 are pruned (exact). Local-source
    edges (plus the self term as synthetic r->r edges) gather from fp16
    ag1_in DURING the AllGather; remote edges gather fp8 rows from the
    fp8 AllGather output (halves collective bytes; gathers are
    descriptor-latency-bound so payload size is free).
  - z = zpre + lrelu(gconv1) accumulated in place in xt; single z @ enc_W.
  - gconv2 source-side partials into the 1024 KO slots, m2 stored in a
    12/4 tile split so the first gather waves overlap m2 production;
    fp16 ReduceScatter returns each core its 128 slots.
  - sel one-hot matrices packed partition-major so each wave's load is one
    contiguous run per partition (avoids SDMA small-packet contention).
  - Readout (gate + softmax + weighted sum + regression) on-core -> [2].
"""

import functools
import numpy as np

import concourse.bacc as bacc
import concourse.mybir as mybir
import concourse.tile as tile
from concourse import bass
from concourse.bass_utils import run_bass_kernel_spmd
from concourse.masks import make_identity

NE, B, KO = 1024, 16, 64
TX, OM, D = 768, 511, 512
N = NE * B
NCORE = 8
R = N // NCORE        # 2048 rows per core
NT = R // 128         # 16 row tiles per core
SLOPE = 0.3
F32 = mybir.dt.float32
F16 = mybir.dt.float16
F8 = mybir.dt.float8e4
I16 = mybir.dt.int16
AX = mybir.AxisListType.X
ALU = mybir.AluOpType
ACTF = mybir.ActivationFunctionType

WAVE = 8  # max gather chunks per dma_gather call
WCOLS = WAVE * 8


def _wave_sizes(C):
    """Two 4-chunk lead waves cut first-data latency; 8-chunk steady state."""
    return [4, 4] + [8] * ((C - 8) // 8)
DEBUG = False
TRACE = False
TRACE_KW = None


# ---------------------------------------------------------------------------
# host-side edge preparation
# ---------------------------------------------------------------------------

def _chunk_edges_per_tile(src, dstl, nch_t):
    """Sort (src->dst_local) into per-destination-tile 128-edge chunks."""
    C = sum(nch_t)
    idx = np.zeros((C, 128), np.int16)
    dstv = np.full((C, 128), -2.0, np.float32)
    t_of = dstl >> 7
    base = 0
    for t, nch in enumerate(nch_t):
        m = t_of == t
        s = src[m]
        d = (dstl[m] - (t << 7)).astype(np.float32)
        n = len(s)
        assert n <= nch * 128, (n, nch)
        full, rem = divmod(n, 128)
        for j in range(full):
            idx[base + j] = s[j * 128:(j + 1) * 128]
            dstv[base + j] = d[j * 128:(j + 1) * 128]
        if rem:
            idx[base + full, :rem] = s[full * 128:]
            dstv[base + full, :rem] = d[full * 128:]
        base += nch
    return idx, dstv


def _wrap_idx_waves(idx_chunks):
    """[C,128] int16 -> [128, C*8] wrapped per dma_gather call."""
    C = idx_chunks.shape[0]
    cols = []
    cur = 0
    for s in _wave_sizes(C):
        lin = idx_chunks[cur:cur + s].reshape(-1)
        cur += s
        cols.append(np.tile(lin.reshape(-1, 16).T, (8, 1)))
    return np.ascontiguousarray(np.concatenate(cols, axis=1))


def _sel_from_dstv(dstv, dt):
    C = dstv.shape[0]
    sel = (dstv[:, :, None] == np.arange(128, dtype=np.float32)[None, None, :])
    sel = sel.astype(dt)          # [C, 128 slot, 128 dst]
    return np.ascontiguousarray(sel.transpose(1, 0, 2).reshape(128, C * 128))


def _pad_w(w, rows, cols):
    out = np.zeros((rows, cols), np.float32)
    out[:w.shape[0], :w.shape[1]] = w
    return out


# ---------------------------------------------------------------------------
# program builder
# ---------------------------------------------------------------------------

@functools.lru_cache(maxsize=4)
def _build(nch1l_t, nch1r_t, nch2a_t, nch2b_t):
    """gconv1 chunks per dst tile split into local-src (gathered from ag1_in
    during AG1) and remote-src sets; gconv2 chunks per slot tile split by
    m2-row half so its gathers overlap m2 production. Totals are multiples
    of WAVE."""
    C1L = sum(nch1l_t)
    C1R = sum(nch1r_t)
    C2A = sum(nch2a_t)
    C2B = sum(nch2b_t)
    W1 = (C1L + C1R) // WAVE
    W2 = (C2A + C2B) // WAVE
    nc = bacc.Bacc("TRN2", num_swdge_queues=4)

    def din(name, shape, dtype=F16):
        return nc.dram_tensor(name, shape, dtype, kind="ExternalInput")

    x_t = din("x_t", [512, R])                  # [x | ko]^T fp16
    pre_t_d = din("pre_t", [512, R])
    ndemb = din("ndemb", [128, 12 * 128])
    # pre-transposed on host: [p, mo, ki, m] = W[ki*128+p, mo*128+m]
    name_W = din("name_W", [128, 6 * TX])
    desc_W = din("desc_W", [128, 6 * TX])
    omic_W = din("omic_W", [512, 512])
    fus_nd = din("fus_nd", [128, 4 * 12 * 128])
    fus_om = din("fus_om", [512, 512])
    ienc_W = din("ienc_W", [512, 512])
    pre_W = din("pre_W", [512, 512])
    enc_W = din("enc_W", [512, 512])
    gate_W1 = din("gate_W1", [512, 512], F32)
    gw2reg = din("gw2reg", [128, 8], F32)
    bias_pf = din("bias_pf", [128, 26], F32)
    bias_rows = din("bias_rows", [96, 512], F32)
    idx1l_d = din("idx1l", [128, (C1L // WAVE) * WCOLS], I16)
    sel1l_d = din("sel1l", [128, C1L * 128], F16)
    idx1r_d = din("idx1r", [128, (C1R // WAVE) * WCOLS], I16)
    sel1r_d = din("sel1r", [128, C1R * 128], F8)
    idx2a_d = din("idx2a", [128, (C2A // WAVE) * WCOLS], I16)
    sel2a_d = din("sel2a", [128, C2A * 128], F16)
    idx2b_d = din("idx2b", [128, (C2B // WAVE) * WCOLS], I16)
    sel2b_d = din("sel2b", [128, C2B * 128], F16)
    out_d = nc.dram_tensor("out", [1, 2], F32, kind="ExternalOutput")

    agnd_in = nc.dram_tensor("agnd_in", [512, 128], F16)
    agnd_out = nc.dram_tensor("agnd_out", [NCORE * 512, 128], F16, addr_space="Shared")
    ag1_in = nc.dram_tensor("ag1_in", [R, 512], F16)
    ag8_in = nc.dram_tensor("ag8_in", [R // 4, 4 * 512], F8)
    ag8_out = nc.dram_tensor("ag8_out", [N // 4, 4 * 512], F8, addr_space="Shared")
    m2_a = nc.dram_tensor("m2_a", [12 * 128, 512], F16)
    m2_b = nc.dram_tensor("m2_b", [4 * 128, 512], F16)
    rs_in = nc.dram_tensor("rs_in", [8 * 32, 4 * 512], F16)
    rs_out = nc.dram_tensor("rs_out", [32, 4 * 512], F16)
    RG = [list(range(NCORE))]

    if DEBUG:
        dbg_cross = nc.dram_tensor("dbg_cross", [512, R], F16, kind="ExternalOutput")
        dbg_m1 = nc.dram_tensor("dbg_m1", [R, 512], F16, kind="ExternalOutput")
        dbg_m2 = nc.dram_tensor("dbg_m2", [R, 512], F16, kind="ExternalOutput")
        dbg_zk = nc.dram_tensor("dbg_zk", [128, 512], F32, kind="ExternalOutput")

    with tile.TileContext(nc) as tc:
        with (
            tc.tile_pool(name="pbig", bufs=8) as pbig,
            tc.tile_pool(name="pmed", bufs=1) as pmed,
            tc.tile_pool(name="pw", bufs=1) as pw,
            tc.tile_pool(name="pg", bufs=1) as pg,
            tc.tile_pool(name="psc", bufs=1) as psc,
            tc.tile_pool(name="pp", bufs=1, space="PSUM") as pp,
        ):
            # ---- ND embeddings first: one contiguous load feeding the
            # first matmuls ----
            emb_all = psc.tile([128, 12, 128], F16, tag="emb", bufs=1)
            nc.sync.dma_start(
                out=emb_all[:].rearrange("p a c -> p (a c)"), in_=ndemb[:])

            # ---- constants ----
            bpf = psc.tile([128, 26], F32, tag="bpf", bufs=1)
            nc.sync.dma_start(out=bpf[:], in_=bias_pf[:])
            brow_g = psc.tile([1, 512], F32, tag="brow_g", bufs=1)
            nc.sync.dma_start(out=brow_g[:], in_=bias_rows[64:65, :])
            ones = psc.tile([1, 512], F32, tag="ones", bufs=1)
            nc.vector.memset(ones[:], 1.0)
            ident = psc.tile([128, 128], F32, tag="ident", bufs=1)
            make_identity(nc, ident[:])
            idxs = {}
            for nm, dd in (("1l", idx1l_d), ("1r", idx1r_d),
                           ("2a", idx2a_d), ("2b", idx2b_d)):
                t_ = psc.tile([128, dd.shape[1]], I16, tag=f"idx{nm}", bufs=1)
                nc.sync.dma_start(out=t_[:], in_=dd[:])
                idxs[nm] = t_

            # ---- ND path (128 entities) — issued first so AG-nd fires early ----
            nd_act = []
            for half in range(2):
                W_d = name_W if half == 0 else desc_W
                embs = [emb_all[:, 6 * half + ki, :] for ki in range(6)]
                for mo in range(6):
                    ps = pp.tile([128, 512], F32, tag="ps_mm", bufs=2, space="PSUM")
                    wstrip = pw.tile([128, 6, 128], F16, tag="wnd6", bufs=3)
                    nc.sync.dma_start(
                        out=wstrip[:].rearrange("p a m -> p (a m)"),
                        in_=W_d[:, 768 * mo:768 * (mo + 1)])
                    for ki in range(6):
                        nc.tensor.matmul(ps[:, :128], lhsT=wstrip[:, ki, :],
                                         rhs=embs[ki],
                                         start=(ki == 0), stop=(ki == 5))
                    a = psc.tile([128, 128], F16, tag="ndact", bufs=12,
                                 name=f"ndact{half}_{mo}")
                    bt = bpf[:, 6 * half + mo:6 * half + mo + 1]
                    tnd = psc.tile([128, 128], F32, tag="tmpnd", bufs=2)
                    nc.vector.tensor_scalar(out=tnd[:], in0=ps[:, :128], scalar1=bt,
                                            scalar2=SLOPE, op0=ALU.add, op1=ALU.mult)
                    nc.vector.tensor_scalar(out=a[:], in0=ps[:, :128], scalar1=bt,
                                            scalar2=None, op0=ALU.add)
                    nc.vector.tensor_tensor(out=a[:], in0=a[:], in1=tnd[:], op=ALU.max)
                    nd_act.append(a)
            for mo in range(4):
                ps = pp.tile([128, 512], F32, tag="ps_mm", bufs=2, space="PSUM")
                wstrip = pw.tile([128, 12, 128], F16, tag="wnd12", bufs=2)
                nc.sync.dma_start(
                    out=wstrip[:].rearrange("p a m -> p (a m)"),
                    in_=fus_nd[:, 1536 * mo:1536 * (mo + 1)])
                for ki in range(12):
                    nc.tensor.matmul(ps[:, :128], lhsT=wstrip[:, ki, :],
                                     rhs=nd_act[ki][:],
                                     start=(ki == 0), stop=(ki == 11))
                r_ = psc.tile([128, 128], F16, tag="ndres", bufs=4, name=f"ndres{mo}")
                nc.vector.tensor_copy(out=r_[:], in_=ps[:, :128])
                nc.sync.dma_start(out=agnd_in[128 * mo:128 * (mo + 1), :], in_=r_[:])
            nc.gpsimd.collective_compute(
                "AllGather", ALU.bypass, replica_groups=RG,
                ins=[agnd_in[:]], outs=[agnd_out[:]])

            # ---- big activations (fp16), loaded behind the ND-path inputs ----
            xt = []
            for k in range(4):
                t = pbig.tile([128, R], F16, tag="bigA", bufs=8, name=f"xt{k}")
                nc.sync.dma_start(out=t[:], in_=x_t[128 * k:128 * (k + 1), :])
                xt.append(t)

            # ---- omic + fus -> cross_c^T (fp16) ----
            womic = [pw.tile([128, 512], F16, tag="wres", bufs=12, name=f"womic{k}")
                     for k in range(4)]
            wfom = [pw.tile([128, 512], F16, tag="wres", bufs=12, name=f"wfom{k}")
                    for k in range(4)]
            for k in range(4):
                nc.sync.dma_start(out=womic[k][:], in_=omic_W[128 * k:128 * (k + 1), :])
                nc.sync.dma_start(out=wfom[k][:], in_=fus_om[128 * k:128 * (k + 1), :])
            # tiled cross_nd landed in SBUF once; folded in via fused vector add
            nd_sb = [pmed.tile([128, 1024], F16, tag="ndsb", bufs=4, name=f"ndsb{k}")
                     for k in range(4)]
            for k in range(4):
                nc.sync.dma_start(
                    out=nd_sb[k][:].rearrange("p (r c) -> p r c", r=NCORE),
                    in_=agnd_out[:].rearrange("(r q p) c -> q p r c",
                                              r=NCORE, q=4)[k])
            cross = [pbig.tile([128, R], F16, tag="bigA", bufs=8, name=f"cross{k}")
                     for k in range(4)]
            for j in range(4):
                sl = slice(512 * j, 512 * (j + 1))
                om_j = []
                for k in range(4):
                    ps = pp.tile([128, 512], F32, tag="ps_mm", bufs=2, space="PSUM")
                    for ki in range(4):
                        nc.tensor.matmul(ps[:], lhsT=womic[ki][:, 128 * k:128 * (k + 1)],
                                         rhs=xt[ki][:, sl], start=(ki == 0), stop=(ki == 3))
                    a = pmed.tile([128, 512], F16, tag="omj", bufs=4)
                    bt = bpf[:, 12 + k:13 + k]
                    tom = pmed.tile([128, 512], F32, tag="tmpom", bufs=2)
                    nc.vector.tensor_scalar(out=tom[:], in0=ps[:], scalar1=bt,
                                            scalar2=SLOPE, op0=ALU.add, op1=ALU.mult)
                    nc.vector.tensor_scalar(out=a[:], in0=ps[:], scalar1=bt,
                                            scalar2=None, op0=ALU.add)
                    nc.vector.tensor_tensor(out=a[:], in0=a[:], in1=tom[:], op=ALU.max)
                    om_j.append(a)
                for k in range(4):
                    ps = pp.tile([128, 512], F32, tag="ps_mm", bufs=2, space="PSUM")
                    for ki in range(4):
                        nc.tensor.matmul(ps[:], lhsT=wfom[ki][:, 128 * k:128 * (k + 1)],
                                         rhs=om_j[ki][:], start=(ki == 0),
                                         stop=(ki == 3))
                    nc.vector.tensor_copy(out=cross[k][:, sl], in_=ps[:])
                    # + tiled cross_nd (fus_b asserted zero) on GpSimd so the
                    # vector/PSUM pipeline never waits on AG-nd
                    e0 = 512 * (j % 2)
                    nc.gpsimd.tensor_tensor(out=cross[k][:, sl],
                                            in0=cross[k][:, sl],
                                            in1=nd_sb[k][:, e0:e0 + 512], op=ALU.add)
            nc.sync.dma_start(out=cross[3][127:128, :], in_=x_t[511:512, :])
            if DEBUG:
                for k in range(4):
                    nc.sync.dma_start(out=dbg_cross[128 * k:128 * (k + 1), :],
                                      in_=cross[k][:])

            # ---- generic gather+scatter ----
            def _bounds(nch_t):
                b = []
                for t_id, nch in enumerate(nch_t):
                    for j in range(nch):
                        b.append((t_id, j == 0, j == nch - 1))
                return b

            def scatter(src_dram, idx_t, sel_d, sel_dt, nchunks, tile_bounds,
                        psum_tag, gbufs_n, sfx=""):
                out_psums = []
                ps = None
                src_ap = src_dram if isinstance(src_dram, bass.AP) else src_dram[:]
                cur = 0
                i = 0
                for w, s in enumerate(_wave_sizes(nchunks)):
                    g = pg.tile([128, WAVE, 512], sel_dt, tag="gath" + sfx,
                                bufs=gbufs_n)
                    nc.gpsimd.dma_gather(
                        g[:, :s, :], src_ap, idx_t[:, 8 * cur:8 * (cur + s)],
                        s * 128, s * 128, 512,
                        single_packet=True, queue_num=w % 4)
                    sw = pg.tile([128, WAVE, 128], sel_dt, tag="selw" + sfx,
                                 bufs=gbufs_n)
                    nc.sync.dma_start(
                        out=sw[:, :s, :].rearrange("p a d -> p (a d)"),
                        in_=sel_d[:, 128 * cur:128 * (cur + s)])
                    for slot in range(s):
                        t_id, first, last = tile_bounds[i]
                        i += 1
                        if first:
                            ps = pp.tile([128, 512], F32, tag=psum_tag, bufs=2,
                                         space="PSUM")
                        nc.tensor.matmul(ps[:], lhsT=sw[:, slot, :],
                                         rhs=g[:, slot, :],
                                         start=first, stop=last)
                        if last:
                            out_psums.append((t_id, ps))
                    cur += s
                return out_psums

            # ---- m1 (row-major fp16) + AG1 ----
            wienc = [pw.tile([128, 512], F16, tag="wres", bufs=12, name=f"wienc{k}")
                     for k in range(4)]
            for k in range(4):
                nc.sync.dma_start(out=wienc[k][:], in_=ienc_W[128 * k:128 * (k + 1), :])
            for t in range(NT):
                tsl = slice(128 * t, 128 * (t + 1))
                ps = pp.tile([128, 512], F32, tag="ps_mm", bufs=2, space="PSUM")
                for ki in range(4):
                    nc.tensor.matmul(ps[:], lhsT=cross[ki][:, tsl], rhs=wienc[ki][:],
                                     start=(ki == 0), stop=(ki == 3))
                h = pmed.tile([128, 512], F16, tag="m1h", bufs=3)
                nc.vector.tensor_copy(out=h[:], in_=ps[:])
                nc.sync.dma_start(out=ag1_in[tsl, :], in_=h[:])
                h8 = pmed.tile([128, 512], F8, tag="m1h8", bufs=3)
                nc.vector.tensor_copy(out=h8[:], in_=ps[:])
                nc.sync.dma_start(
                    out=ag8_in[32 * t:32 * (t + 1), :].rearrange(
                        "a (b f) -> (a b) f", b=4),
                    in_=h8[:])
            nc.gpsimd.collective_compute(
                "AllGather", ALU.bypass, replica_groups=RG,
                ins=[ag8_in[:]], outs=[ag8_out[:]])
            # ---- zpre (in place on xt; pre_b asserted zero). The pre_t
            # streams are prefetched in full so the scheduler sees zpre as
            # ready work for the AG1 window (before the gather-fed matmuls).
            wpre = [pw.tile([128, 512], F16, tag="wres", bufs=12, name=f"wpre{k}")
                    for k in range(4)]
            for k in range(4):
                nc.sync.dma_start(out=wpre[k][:], in_=pre_W[128 * k:128 * (k + 1), :])
            pre_all = {}
            for j in range(4):
                for ki in range(4):
                    s = pmed.tile([128, 512], F16, tag="prestream", bufs=16)
                    nc.sync.dma_start(
                        out=s[:],
                        in_=pre_t_d[128 * ki:128 * (ki + 1),
                                    slice(512 * j, 512 * (j + 1))])
                    pre_all[j, ki] = s
            for j in range(4):
                sl = slice(512 * j, 512 * (j + 1))
                pre_j = [pre_all[j, ki] for ki in range(4)]
                for k in range(4):
                    ps = pp.tile([128, 512], F32, tag="ps_mm", bufs=2, space="PSUM")
                    for ki in range(4):
                        nc.tensor.matmul(ps[:], lhsT=wpre[ki][:, 128 * k:128 * (k + 1)],
                                         rhs=pre_j[ki][:], start=(ki == 0),
                                         stop=(ki == 3))
                    nc.vector.tensor_tensor(out=xt[k][:, sl], in0=xt[k][:, sl],
                                            in1=ps[:], op=ALU.add)

            # ---- gconv1 local-src edges: gathered from ag1_in DURING AG1 ----
            uacc = [pmed.tile([128, 512], F16, tag="uacc", bufs=NT, name=f"uacc{t}")
                    for t in range(NT)]
            seg1l = scatter(ag1_in, idxs["1l"], sel1l_d, F16, C1L,
                            _bounds(nch1l_t), "ps_seg", 2)
            for t_id, ps in seg1l:
                nc.vector.tensor_copy(out=uacc[t_id][:], in_=ps[:])

            # ---- enc weights (used by fused z @ enc_W after the scatter) ----
            wenc = [pw.tile([128, 512], F16, tag="wres", bufs=12, name=f"wenc{k}")
                    for k in range(4)]
            for k in range(4):
                nc.sync.dma_start(out=wenc[k][:], in_=enc_W[128 * k:128 * (k + 1), :])

            ag8_rows = ag8_out[:].rearrange("a (b f) -> (a b) f", b=4)
            seg1r = scatter(ag8_rows, idxs["1r"], sel1r_d, F8, C1R,
                            _bounds(nch1r_t), "ps_seg", 4, sfx="8")

            # ---- z^T = zpre^T + lrelu(gconv1)^T, accumulated in place in xt ----
            for q in range(4):
                pst = [pp.tile([128, 512], F32, tag="ps_ut", bufs=4, space="PSUM",
                               name=f"pst{q}_{k_}") for k_ in range(4)]
                for tt in range(4):
                    t_id, ps = seg1r[4 * q + tt]
                    useg = pmed.tile([128, 512], F32, tag="useg", bufs=3)
                    nc.vector.tensor_tensor(out=useg[:], in0=ps[:],
                                            in1=uacc[t_id][:], op=ALU.add)
                    for k in range(4):
                        nc.tensor.transpose(
                            out=pst[k][:, 128 * tt:128 * (tt + 1)],
                            in_=useg[:, 128 * k:128 * (k + 1)], identity=ident[:])
                for k in range(4):
                    sl = slice(512 * q, 512 * (q + 1))
                    tmp = pmed.tile([128, 512], F32, tag="lrtmp", bufs=2)
                    nc.vector.tensor_scalar(
                        out=tmp[:], in0=pst[k][:], scalar1=bpf[:, 16 + k:17 + k],
                        scalar2=SLOPE, op0=ALU.add, op1=ALU.mult)
                    u16 = pmed.tile([128, 512], F16, tag="u16", bufs=2)
                    nc.vector.tensor_scalar(
                        out=u16[:], in0=pst[k][:], scalar1=bpf[:, 16 + k:17 + k],
                        scalar2=None, op0=ALU.add)
                    nc.vector.tensor_tensor(
                        out=u16[:], in0=u16[:], in1=tmp[:], op=ALU.max)
                    nc.vector.tensor_tensor(
                        out=xt[k][:, sl], in0=xt[k][:, sl], in1=u16[:], op=ALU.add)

            # ---- m2 = z @ enc_W (fp16, stays local; stored in two halves so
            # gconv2's first gather waves overlap the second half) ----
            for t in range(NT):
                tsl = slice(128 * t, 128 * (t + 1))
                ps = pp.tile([128, 512], F32, tag="ps_mm", bufs=2, space="PSUM")
                for ki in range(4):
                    nc.tensor.matmul(ps[:], lhsT=xt[ki][:, tsl], rhs=wenc[ki][:],
                                     start=(ki == 0), stop=(ki == 3))
                h = pmed.tile([128, 512], F16, tag="m2h", bufs=3)
                nc.vector.tensor_copy(out=h[:], in_=ps[:])
                if t < 12:
                    nc.sync.dma_start(out=m2_a[tsl, :], in_=h[:])
                else:
                    nc.sync.dma_start(
                        out=m2_b[128 * (t - 12):128 * (t - 11), :], in_=h[:])

            # ---- gconv2: source-side partials over 1024 slots + ReduceScatter ----
            acc2 = [pmed.tile([128, 512], F16, tag="acc2", bufs=8, name=f"acc2{t}")
                    for t in range(8)]
            seg2a = scatter(m2_a, idxs["2a"], sel2a_d, F16, C2A,
                            _bounds(nch2a_t), "ps_seg", 2)
            for t_id, ps in seg2a:
                nc.vector.tensor_copy(out=acc2[t_id][:], in_=ps[:])
            seg2b = scatter(m2_b, idxs["2b"], sel2b_d, F16, C2B,
                            _bounds(nch2b_t), "ps_seg", 2)
            for t_id, ps in seg2b:
                pc = pmed.tile([128, 512], F16, tag="m2h", bufs=3)
                nc.vector.tensor_tensor(out=pc[:], in0=ps[:], in1=acc2[t_id][:],
                                        op=ALU.add)
                nc.sync.dma_start(
                    out=rs_in[32 * t_id:32 * (t_id + 1), :].rearrange(
                        "a (b f) -> (a b) f", b=4),
                    in_=pc[:])
            nc.gpsimd.collective_compute(
                "ReduceScatter", ALU.add, replica_groups=RG,
                ins=[rs_in[:]], outs=[rs_out[:]])

            # ---- zk^T + readout ----
            zkr16 = pmed.tile([128, 512], F16, tag="m2h", bufs=3)
            nc.sync.dma_start(
                out=zkr16[:],
                in_=rs_out[:].rearrange("a (b f) -> (a b) f", b=4))
            zkr = pmed.tile([128, 512], F32, tag="useg", bufs=3)
            nc.vector.tensor_copy(out=zkr[:], in_=zkr16[:])
            if DEBUG:
                nc.sync.dma_start(out=dbg_zk[:], in_=zkr[:])
            ps_zt = pp.tile([128, 512], F32, tag="ps_ut", bufs=4, space="PSUM")
            for k in range(4):
                nc.tensor.transpose(out=ps_zt[:, 128 * k:128 * (k + 1)],
                                    in_=zkr[:, 128 * k:128 * (k + 1)], identity=ident[:])
            zkt = pmed.tile([128, 512], F32, tag="zkt", bufs=1)
            for k in range(4):
                sl = slice(128 * k, 128 * (k + 1))
                tmp = pmed.tile([128, 128], F32, tag="lrtmp2", bufs=2)
                nc.vector.tensor_scalar(
                    out=tmp[:], in0=ps_zt[:, sl], scalar1=bpf[:, 20 + k:21 + k],
                    scalar2=SLOPE, op0=ALU.add, op1=ALU.mult)
                nc.vector.tensor_scalar(
                    out=zkt[:, sl], in0=ps_zt[:, sl], scalar1=bpf[:, 20 + k:21 + k],
                    scalar2=None, op0=ALU.add)
                nc.vector.tensor_tensor(
                    out=zkt[:, sl], in0=zkt[:, sl], in1=tmp[:], op=ALU.max)

            wg1 = [pw.tile([128, 512], F32, tag="wres32", bufs=4, name=f"wg1{k}")
                   for k in range(4)]
            for k in range(4):
                nc.sync.dma_start(out=wg1[k][:], in_=gate_W1[128 * k:128 * (k + 1), :])
            w2r = psc.tile([128, 8], F32, tag="w2r", bufs=1)
            nc.sync.dma_start(out=w2r[:], in_=gw2reg[:])
            s1t = pmed.tile([128, 512], F32, tag="s1t", bufs=1)
            for ko_ in range(4):
                ps = pp.tile([128, 512], F32, tag="ps_mm", bufs=2, space="PSUM")
                for ki in range(4):
                    nc.tensor.matmul(ps[:, :128],
                                     lhsT=wg1[ki][:, 128 * ko_:128 * (ko_ + 1)],
                                     rhs=zkt[:, 128 * ki:128 * (ki + 1)],
                                     start=(ki == 0), stop=False)
                nc.tensor.matmul(ps[:, :128],
                                 lhsT=brow_g[:, 128 * ko_:128 * (ko_ + 1)],
                                 rhs=ones[:, :128], start=False, stop=True)
                nc.scalar.activation(s1t[:, 128 * ko_:128 * (ko_ + 1)], ps[:, :128],
                                     ACTF.Tanh)
            ps_sc = pp.tile([128, 512], F32, tag="ps_mm", bufs=2, space="PSUM")
            for ki in range(4):
                nc.tensor.matmul(ps_sc[:1, :128], lhsT=w2r[:, 2 * ki:2 * ki + 1],
                                 rhs=s1t[:, 128 * ki:128 * (ki + 1)],
                                 start=(ki == 0), stop=(ki == 3))
            ps_tr = pp.tile([128, 512], F32, tag="ps_seg", bufs=2, space="PSUM")
            for ki in range(4):
                nc.tensor.matmul(ps_tr[:1, :128], lhsT=w2r[:, 2 * ki + 1:2 * ki + 2],
                                 rhs=zkt[:, 128 * ki:128 * (ki + 1)],
                                 start=(ki == 0), stop=(ki == 3))
            erow = psc.tile([1, 128], F32, tag="erow", bufs=1)
            nc.scalar.activation(erow[:], ps_sc[:1, :128], ACTF.Exp,
                                 bias=bpf[:1, 24:25])
            etrow = psc.tile([1, 128], F32, tag="etrow", bufs=1)
            nc.vector.tensor_tensor(out=etrow[:], in0=erow[:], in1=ps_tr[:1, :128],
                                    op=ALU.mult)
            sums = psc.tile([1, 4], F32, tag="sums", bufs=1)
            nc.vector.tensor_reduce(out=sums[:, 0:2],
                                    in_=etrow[:].rearrange("p (g x) -> p g x", g=2),
                                    axis=AX, op=ALU.add)
            nc.vector.tensor_reduce(out=sums[:, 2:4],
                                    in_=erow[:].rearrange("p (g x) -> p g x", g=2),
                                    axis=AX, op=ALU.add)
            res = psc.tile([1, 4], F32, tag="res", bufs=1)
            nc.vector.reciprocal(out=res[:, 2:4], in_=sums[:, 2:4])
            nc.vector.tensor_tensor(out=res[:, 0:2], in0=sums[:, 0:2],
                                    in1=res[:, 2:4], op=ALU.mult)
            nc.vector.tensor_scalar(out=res[:, 0:2], in0=res[:, 0:2],
                                    scalar1=bpf[:1, 25:26], scalar2=None, op0=ALU.add)
            nc.sync.dma_start(out=out_d[:], in_=res[:, 0:2])

    nc.compile()
    return nc


def _ensure_ntff_hook():
    """Inject antenv.axon_hooks (absent in this image) so trace=True works."""
    import sys, types
    try:
        from antenv.axon_hooks import get_axon_ntff_profile_hook  # noqa
        return
    except ImportError:
        pass
    import antenv
    mod = types.ModuleType("antenv.axon_hooks")
    _state = {"hook": None}
    mod.set_axon_ntff_profile_hook = lambda h: _state.__setitem__("hook", h)
    mod.get_axon_ntff_profile_hook = lambda: _state["hook"]
    sys.modules["antenv.axon_hooks"] = mod
    antenv.axon_hooks = mod
    from trn_agent_boot.trn_boot import _ntff_profile_via_ctypes
    mod.set_axon_ntff_profile_hook(
        _ntff_profile_via_ctypes("/opt/axon/libaxon_pjrt.so"))


# ---------------------------------------------------------------------------
# host wrapper
# ---------------------------------------------------------------------------

def kernel(**inputs):
    f32 = lambda k: np.asarray(inputs[k], np.float32)
    x = f32("x"); pre_x = f32("pre_x")
    edge_index = np.asarray(inputs["edge_index"], np.int64)
    internal_edge_index = np.asarray(inputs["internal_edge_index"], np.int64)
    name_emb = f32("name_embeddings"); desc_emb = f32("desc_embeddings")
    ko_mask = np.asarray(inputs["ko_mask"], np.int64)
    bkm = np.asarray(inputs["batch_ko_masks"], np.int64)
    name_W = f32("name_W"); name_b = f32("name_b")
    desc_W = f32("desc_W"); desc_b = f32("desc_b")
    omic_W = f32("omic_W"); omic_b = f32("omic_b")
    fus_W = f32("fus_W"); fus_b = f32("fus_b")
    pre_W = f32("pre_W"); pre_b = f32("pre_b")
    ienc_W = f32("ienc_W"); ienc_b = f32("ienc_b")
    enc_W = f32("enc_W"); enc_b = f32("enc_b")
    gate_W1 = f32("gate_W1"); gate_b1 = f32("gate_b1")
    gate_W2 = f32("gate_W2"); gate_b2 = f32("gate_b2")
    reg_W = f32("reg_W"); reg_b = f32("reg_b")

    assert not fus_b.any() and not pre_b.any(), \
        "nonzero fus_b/pre_b not supported by this build"

    ko_feat = np.zeros(N, np.float32)
    ko_feat[ko_mask] = 1.0

    # ---- gconv2: source-sharded edges into the 1024 global KO slots ----
    slot_row = (bkm + np.arange(B)[:, None] * NE).reshape(-1)   # [1024]
    row2slots = {}
    for s_, r_ in enumerate(slot_row):
        row2slots.setdefault(int(r_), []).append(s_)
    def _pad_last(nch_t):
        nch_t[-1] += (-int(nch_t.sum())) % WAVE
        return tuple(int(v) for v in nch_t)

    s2_all, d2_all = edge_index[0], edge_index[1]
    m2mask = np.isin(d2_all, slot_row)
    per_core_2a = []   # sources in local rows [0, R/2)
    per_core_2b = []   # sources in local rows [R/2, R)
    needed = []        # per-core local rows whose z/m2 is actually consumed
    nch2a_t = np.ones(8, np.int64)
    nch2b_t = np.ones(8, np.int64)
    for c in range(NCORE):
        lo, hi = R * c, R * (c + 1)
        ss, ds = [], []
        for r_, sl_ in row2slots.items():
            if lo <= r_ < hi:
                for s_ in sl_:
                    ss.append(r_ - lo); ds.append(s_)
        mm = m2mask & (s2_all >= lo) & (s2_all < hi)
        for u, v in zip(s2_all[mm], d2_all[mm]):
            for s_ in row2slots[int(v)]:
                ss.append(int(u) - lo); ds.append(s_)
        src = np.array(ss, np.int64); dstl = np.array(ds, np.int64)
        nd = np.zeros(R, bool)
        nd[src] = True
        needed.append(nd)
        ha = src < 12 * 128
        per_core_2a.append((src[ha], dstl[ha]))
        per_core_2b.append((src[~ha] - 12 * 128, dstl[~ha]))
        nch2a_t = np.maximum(nch2a_t, -(-np.bincount(dstl[ha] >> 7, minlength=8) // 128))
        nch2b_t = np.maximum(nch2b_t, -(-np.bincount(dstl[~ha] >> 7, minlength=8) // 128))
    nch2a_t = _pad_last(nch2a_t)
    nch2b_t = _pad_last(nch2b_t)

    # ---- gconv1 edges (dst-sharded; self term added from local m1h).
    # Edges whose dst row never feeds gconv2 (not a slot row, not a source of
    # a slot edge) are dropped: their z rows are never read. Edges with a
    # LOCAL source are gathered from ag1_in during AG1. ----
    s1_all, d1_all = internal_edge_index[0], internal_edge_index[1]
    per_core_1l = []
    per_core_1r = []
    nch1l_t = np.ones(NT, np.int64)
    nch1r_t = np.ones(NT, np.int64)
    for c in range(NCORE):
        lo, hi = R * c, R * (c + 1)
        m = (d1_all >= lo) & (d1_all < hi)
        s1 = s1_all[m]; d1l = d1_all[m] - lo
        keep = needed[c][d1l]
        s1 = s1[keep]; d1l = d1l[keep]
        isloc = (s1 >= lo) & (s1 < hi)
        # the gconv self term rides the local pass as synthetic (r -> r) edges
        selfr = np.nonzero(needed[c])[0]
        ls = np.concatenate([s1[isloc] - lo, selfr])
        ld = np.concatenate([d1l[isloc], selfr])
        per_core_1l.append((ls, ld))
        per_core_1r.append((s1[~isloc], d1l[~isloc]))
        nch1l_t = np.maximum(
            nch1l_t, -(-np.bincount(ld >> 7, minlength=NT) // 128))
        nch1r_t = np.maximum(
            nch1r_t, -(-np.bincount(d1l[~isloc] >> 7, minlength=NT) // 128))
    nch1l_t = _pad_last(nch1l_t)
    nch1r_t = _pad_last(nch1r_t)

    nc = _build(nch1l_t, nch1r_t, nch2a_t, nch2b_t)

    import ml_dtypes
    f16 = np.float16
    f8 = ml_dtypes.float8_e4m3
    omic_Wp = _pad_w(omic_W, 512, 512)
    fus_ndp = _pad_w(fus_W[:2 * TX], 2 * TX, 512)
    fus_omp = _pad_w(fus_W[2 * TX:], 512, 512)
    # [p, mo, ki, m] = W[ki*128+p, mo*128+m] so wstrip loads are contiguous
    name_Wr = np.ascontiguousarray(
        name_W.reshape(6, 128, 6, 128).transpose(1, 2, 0, 3).reshape(128, 6 * TX))
    desc_Wr = np.ascontiguousarray(
        desc_W.reshape(6, 128, 6, 128).transpose(1, 2, 0, 3).reshape(128, 6 * TX))
    fus_ndr = np.ascontiguousarray(
        fus_ndp.reshape(12, 128, 4, 128).transpose(1, 2, 0, 3).reshape(128, 6144))
    bias_pf = np.zeros((128, 26), np.float32)
    bias_pf[:, 0:6] = name_b.reshape(6, 128).T
    bias_pf[:, 6:12] = desc_b.reshape(6, 128).T
    bias_pf[:, 12:16] = _pad_w(omic_b[:, None], 512, 1).reshape(4, 128).T
    bias_pf[:, 16:20] = ienc_b.reshape(4, 128).T
    bias_pf[:, 20:24] = enc_b.reshape(4, 128).T
    bias_pf[:, 24] = float(gate_b2.reshape(-1)[0])
    bias_pf[:, 25] = float(reg_b.reshape(-1)[0])
    bias_rows = np.zeros((96, 512), np.float32)
    bias_rows[64, :] = gate_b1
    gw2 = np.concatenate([gate_W2, reg_W], axis=1).astype(np.float32)
    gw2 = np.ascontiguousarray(
        gw2.reshape(4, 128, 2).transpose(1, 0, 2).reshape(128, 8))

    shared = dict(
        name_W=name_Wr.astype(f16), desc_W=desc_Wr.astype(f16),
        omic_W=omic_Wp.astype(f16), fus_nd=fus_ndr.astype(f16),
        fus_om=fus_omp.astype(f16), ienc_W=ienc_W.astype(f16),
        pre_W=pre_W.astype(f16), enc_W=enc_W.astype(f16),
        gate_W1=gate_W1, gw2reg=gw2, bias_pf=bias_pf, bias_rows=bias_rows,
    )

    in_maps = []
    for c in range(NCORE):
        lo, hi = R * c, R * (c + 1)
        x_t = np.concatenate([x[lo:hi].T, ko_feat[None, lo:hi]], 0)
        pre_t = np.concatenate([pre_x[lo:hi].T, ko_feat[None, lo:hi]], 0)
        ndemb = np.concatenate(
            [name_emb[128 * c:128 * (c + 1)].T, desc_emb[128 * c:128 * (c + 1)].T], 0)
        ndemb = ndemb.reshape(12, 128, 128).transpose(1, 0, 2).reshape(128, 12 * 128)
        i1l, dv1l = _chunk_edges_per_tile(*per_core_1l[c], nch1l_t)
        i1r, dv1r = _chunk_edges_per_tile(*per_core_1r[c], nch1r_t)
        i2a, dv2a = _chunk_edges_per_tile(*per_core_2a[c], nch2a_t)
        i2b, dv2b = _chunk_edges_per_tile(*per_core_2b[c], nch2b_t)
        in_maps.append(dict(
            x_t=np.ascontiguousarray(x_t).astype(f16),
            pre_t=np.ascontiguousarray(pre_t).astype(f16),
            ndemb=np.ascontiguousarray(ndemb).astype(f16),
            idx1l=_wrap_idx_waves(i1l), sel1l=_sel_from_dstv(dv1l, f16),
            idx1r=_wrap_idx_waves(i1r), sel1r=_sel_from_dstv(dv1r, f8),
            idx2a=_wrap_idx_waves(i2a), sel2a=_sel_from_dstv(dv2a, f16),
            idx2b=_wrap_idx_waves(i2b), sel2b=_sel_from_dstv(dv2b, f16),
            **shared,
        ))

    if TRACE:
        _ensure_ntff_hook()
    res = run_bass_kernel_spmd(nc, in_maps, core_ids=list(range(NCORE)),
                               trace=TRACE, **(TRACE_KW or {}))
    kernel._last = res
    out = np.zeros(B, np.float32)
    for c in range(NCORE):
        out[2 * c:2 * c + 2] = res.results[c]["out"][0]
    return out



# revision 51
# speedup vs baseline: 1.0530x; 1.0530x over previous
"""Trainium2 Bass kernel for nn_MOTASG_KO_Reg (ragged graph-conv KO regression).

Strategy (8 NeuronCores, data-parallel over node rows):
  - N=16384 nodes = 16 batch samples x 1024 entities. Core c owns rows
    [2048c, 2048c+2048) = batch samples 2c, 2c+1.
  - Activations kept feature-major ("transposed", [feat, rows]) on chip so
    every linear is a native PE matmul (fp16 operands, fp32 PSUM).
  - name/desc path computed once on 128 entities/core, AllGathered, folded
    into cross via a fused vector add from an SBUF copy.
  - gconv1 segment-sum via dma_gather + one-hot scatter matmuls in PSUM.
    Edges whose dst never feeds gconv2 are pruned (exact). Local-source
    edges (plus the self term as synthetic r->r edges) gather from fp16
    ag1_in DURING the AllGather; remote edges gather fp8 rows from the
    fp8 AllGather output (halves collective bytes; gathers are
    descriptor-latency-bound so payload size is free).
  - z = zpre + lrelu(gconv1) accumulated in place in xt; single z @ enc_W.
  - gconv2 source-side partials into the 1024 KO slots, m2 stored in a
    12/4 tile split so the first gather waves overlap m2 production;
    fp16 ReduceScatter returns each core its 128 slots.
  - sel one-hot matrices packed partition-major so each wave's load is one
    contiguous run per partition (avoids SDMA small-packet contention).
  - Readout (gate + softmax + weighted sum + regression) on-core -> [2].
"""

import functools
import numpy as np

import concourse.bacc as bacc
import concourse.mybir as mybir
import concourse.tile as tile
from concourse import bass
from concourse.bass_utils import run_bass_kernel_spmd
from concourse.masks import make_identity

NE, B, KO = 1024, 16, 64
TX, OM, D = 768, 511, 512
N = NE * B
NCORE = 8
R = N // NCORE        # 2048 rows per core
NT = R // 128         # 16 row tiles per core
SLOPE = 0.3
F32 = mybir.dt.float32
F16 = mybir.dt.float16
F8 = mybir.dt.float8e4
I16 = mybir.dt.int16
AX = mybir.AxisListType.X
ALU = mybir.AluOpType
ACTF = mybir.ActivationFunctionType

WAVE = 8  # max gather chunks per dma_gather call
WCOLS = WAVE * 8


def _wave_sizes(C):
    """Two 4-chunk lead waves cut first-data latency; 8-chunk steady state."""
    return [4, 4] + [8] * ((C - 8) // 8)
DEBUG = False
TRACE = False
TRACE_KW = None


# ---------------------------------------------------------------------------
# host-side edge preparation
# ---------------------------------------------------------------------------

def _chunk_edges_per_tile(src, dstl, nch_t):
    """Sort (src->dst_local) into per-destination-tile 128-edge chunks."""
    C = sum(nch_t)
    idx = np.zeros((C, 128), np.int16)
    dstv = np.full((C, 128), -2.0, np.float32)
    t_of = dstl >> 7
    base = 0
    for t, nch in enumerate(nch_t):
        m = t_of == t
        s = src[m]
        d = (dstl[m] - (t << 7)).astype(np.float32)
        n = len(s)
        assert n <= nch * 128, (n, nch)
        full, rem = divmod(n, 128)
        for j in range(full):
            idx[base + j] = s[j * 128:(j + 1) * 128]
            dstv[base + j] = d[j * 128:(j + 1) * 128]
        if rem:
            idx[base + full, :rem] = s[full * 128:]
            dstv[base + full, :rem] = d[full * 128:]
        base += nch
    return idx, dstv


def _wrap_idx_waves(idx_chunks):
    """[C,128] int16 -> [128, C*8] wrapped per dma_gather call."""
    C = idx_chunks.shape[0]
    cols = []
    cur = 0
    for s in _wave_sizes(C):
        lin = idx_chunks[cur:cur + s].reshape(-1)
        cur += s
        cols.append(np.tile(lin.reshape(-1, 16).T, (8, 1)))
    return np.ascontiguousarray(np.concatenate(cols, axis=1))


def _sel_from_dstv(dstv, dt):
    C = dstv.shape[0]
    sel = (dstv[:, :, None] == np.arange(128, dtype=np.float32)[None, None, :])
    sel = sel.astype(dt)          # [C, 128 slot, 128 dst]
    return np.ascontiguousarray(sel.transpose(1, 0, 2).reshape(128, C * 128))


def _pad_w(w, rows, cols):
    out = np.zeros((rows, cols), np.float32)
    out[:w.shape[0], :w.shape[1]] = w
    return out


# ---------------------------------------------------------------------------
# program builder
# ---------------------------------------------------------------------------

@functools.lru_cache(maxsize=4)
def _build(nch1l_t, nch1r_t, nch2a_t, nch2b_t):
    """gconv1 chunks per dst tile split into local-src (gathered from ag1_in
    during AG1) and remote-src sets; gconv2 chunks per slot tile split by
    m2-row half so its gathers overlap m2 production. Totals are multiples
    of WAVE."""
    C1L = sum(nch1l_t)
    C1R = sum(nch1r_t)
    C2A = sum(nch2a_t)
    C2B = sum(nch2b_t)
    W1 = (C1L + C1R) // WAVE
    W2 = (C2A + C2B) // WAVE
    nc = bacc.Bacc("TRN2", num_swdge_queues=4)

    def din(name, shape, dtype=F16):
        return nc.dram_tensor(name, shape, dtype, kind="ExternalInput")

    x_t = din("x_t", [512, R])                  # [x | ko]^T fp16
    pre_t_d = din("pre_t", [512, R])
    ndemb = din("ndemb", [128, 12 * 128])
    # pre-transposed on host: [p, mo, ki, m] = W[ki*128+p, mo*128+m]
    name_W = din("name_W", [128, 6 * TX])
    desc_W = din("desc_W", [128, 6 * TX])
    omic_W = din("omic_W", [512, 512])
    fus_nd = din("fus_nd", [128, 4 * 12 * 128])
    fus_om = din("fus_om", [512, 512])
    ienc_W = din("ienc_W", [512, 512])
    pre_W = din("pre_W", [512, 512])
    enc_W = din("enc_W", [512, 512])
    gate_W1 = din("gate_W1", [512, 512], F32)
    gw2reg = din("gw2reg", [128, 8], F32)
    bias_pf = din("bias_pf", [128, 26], F32)
    bias_rows = din("bias_rows", [96, 512], F32)
    idx1l_d = din("idx1l", [128, (C1L // WAVE) * WCOLS], I16)
    sel1l_d = din("sel1l", [128, C1L * 128], F16)
    idx1r_d = din("idx1r", [128, (C1R // WAVE) * WCOLS], I16)
    sel1r_d = din("sel1r", [128, C1R * 128], F8)
    idx2a_d = din("idx2a", [128, (C2A // WAVE) * WCOLS], I16)
    sel2a_d = din("sel2a", [128, C2A * 128], F16)
    idx2b_d = din("idx2b", [128, (C2B // WAVE) * WCOLS], I16)
    sel2b_d = din("sel2b", [128, C2B * 128], F16)
    out_d = nc.dram_tensor("out", [1, 2], F32, kind="ExternalOutput")

    agnd_in = nc.dram_tensor("agnd_in", [512, 128], F16)
    agnd_out = nc.dram_tensor("agnd_out", [NCORE * 512, 128], F16, addr_space="Shared")
    ag1_in = nc.dram_tensor("ag1_in", [R, 512], F16)
    ag8_in = nc.dram_tensor("ag8_in", [R // 4, 4 * 512], F8)
    ag8_out = nc.dram_tensor("ag8_out", [N // 4, 4 * 512], F8, addr_space="Shared")
    m2_a = nc.dram_tensor("m2_a", [12 * 128, 512], F16)
    m2_b = nc.dram_tensor("m2_b", [4 * 128, 512], F16)
    rs_in = nc.dram_tensor("rs_in", [8 * 32, 4 * 512], F16)
    rs_out = nc.dram_tensor("rs_out", [32, 4 * 512], F16)
    RG = [list(range(NCORE))]

    if DEBUG:
        dbg_cross = nc.dram_tensor("dbg_cross", [512, R], F16, kind="ExternalOutput")
        dbg_m1 = nc.dram_tensor("dbg_m1", [R, 512], F16, kind="ExternalOutput")
        dbg_m2 = nc.dram_tensor("dbg_m2", [R, 512], F16, kind="ExternalOutput")
        dbg_zk = nc.dram_tensor("dbg_zk", [128, 512], F32, kind="ExternalOutput")

    with tile.TileContext(nc) as tc:
        with (
            tc.tile_pool(name="pbig", bufs=8) as pbig,
            tc.tile_pool(name="pmed", bufs=1) as pmed,
            tc.tile_pool(name="pw", bufs=1) as pw,
            tc.tile_pool(name="pg", bufs=1) as pg,
            tc.tile_pool(name="psc", bufs=1) as psc,
            tc.tile_pool(name="pp", bufs=1, space="PSUM") as pp,
        ):
            # ---- ND embeddings first: one contiguous load feeding the
            # first matmuls ----
            emb_all = psc.tile([128, 12, 128], F16, tag="emb", bufs=1)
            nc.sync.dma_start(
                out=emb_all[:].rearrange("p a c -> p (a c)"), in_=ndemb[:])

            # ---- constants ----
            bpf = psc.tile([128, 26], F32, tag="bpf", bufs=1)
            nc.sync.dma_start(out=bpf[:], in_=bias_pf[:])
            brow_g = psc.tile([1, 512], F32, tag="brow_g", bufs=1)
            nc.sync.dma_start(out=brow_g[:], in_=bias_rows[64:65, :])
            ones = psc.tile([1, 512], F32, tag="ones", bufs=1)
            nc.vector.memset(ones[:], 1.0)
            ident = psc.tile([128, 128], F32, tag="ident", bufs=1)
            make_identity(nc, ident[:])
            idxs = {}
            for nm, dd in (("1l", idx1l_d), ("1r", idx1r_d),
                           ("2a", idx2a_d), ("2b", idx2b_d)):
                t_ = psc.tile([128, dd.shape[1]], I16, tag=f"idx{nm}", bufs=1)
                nc.sync.dma_start(out=t_[:], in_=dd[:])
                idxs[nm] = t_

            # ---- ND path (128 entities) — issued first so AG-nd fires early ----
            nd_act = []
            for half in range(2):
                W_d = name_W if half == 0 else desc_W
                embs = [emb_all[:, 6 * half + ki, :] for ki in range(6)]
                for mo in range(6):
                    ps = pp.tile([128, 512], F32, tag="ps_mm", bufs=2, space="PSUM")
                    wstrip = pw.tile([128, 6, 128], F16, tag="wnd6", bufs=3)
                    nc.sync.dma_start(
                        out=wstrip[:].rearrange("p a m -> p (a m)"),
                        in_=W_d[:, 768 * mo:768 * (mo + 1)])
                    for ki in range(6):
                        nc.tensor.matmul(ps[:, :128], lhsT=wstrip[:, ki, :],
                                         rhs=embs[ki],
                                         start=(ki == 0), stop=(ki == 5))
                    a = psc.tile([128, 128], F16, tag="ndact", bufs=12,
                                 name=f"ndact{half}_{mo}")
                    bt = bpf[:, 6 * half + mo:6 * half + mo + 1]
                    tnd = psc.tile([128, 128], F32, tag="tmpnd", bufs=2)
                    nc.vector.tensor_scalar(out=tnd[:], in0=ps[:, :128], scalar1=bt,
                                            scalar2=SLOPE, op0=ALU.add, op1=ALU.mult)
                    nc.vector.tensor_scalar(out=a[:], in0=ps[:, :128], scalar1=bt,
                                            scalar2=None, op0=ALU.add)
                    nc.vector.tensor_tensor(out=a[:], in0=a[:], in1=tnd[:], op=ALU.max)
                    nd_act.append(a)
            for mo in range(4):
                ps = pp.tile([128, 512], F32, tag="ps_mm", bufs=2, space="PSUM")
                wstrip = pw.tile([128, 12, 128], F16, tag="wnd12", bufs=2)
                nc.sync.dma_start(
                    out=wstrip[:].rearrange("p a m -> p (a m)"),
                    in_=fus_nd[:, 1536 * mo:1536 * (mo + 1)])
                for ki in range(12):
                    nc.tensor.matmul(ps[:, :128], lhsT=wstrip[:, ki, :],
                                     rhs=nd_act[ki][:],
                                     start=(ki == 0), stop=(ki == 11))
                r_ = psc.tile([128, 128], F16, tag="ndres", bufs=4, name=f"ndres{mo}")
                nc.vector.tensor_copy(out=r_[:], in_=ps[:, :128])
                nc.sync.dma_start(out=agnd_in[128 * mo:128 * (mo + 1), :], in_=r_[:])
            nc.gpsimd.collective_compute(
                "AllGather", ALU.bypass, replica_groups=RG,
                ins=[agnd_in[:]], outs=[agnd_out[:]])

            # ---- big activations (fp16), loaded behind the ND-path inputs ----
            xt = []
            for k in range(4):
                t = pbig.tile([128, R], F16, tag="bigA", bufs=8, name=f"xt{k}")
                nc.sync.dma_start(out=t[:], in_=x_t[128 * k:128 * (k + 1), :])
                xt.append(t)

            # ---- omic + fus -> cross_c^T (fp16) ----
            womic = [pw.tile([128, 512], F16, tag="wres", bufs=12, name=f"womic{k}")
                     for k in range(4)]
            wfom = [pw.tile([128, 512], F16, tag="wres", bufs=12, name=f"wfom{k}")
                    for k in range(4)]
            for k in range(4):
                nc.sync.dma_start(out=womic[k][:], in_=omic_W[128 * k:128 * (k + 1), :])
                nc.sync.dma_start(out=wfom[k][:], in_=fus_om[128 * k:128 * (k + 1), :])
            # tiled cross_nd landed in SBUF once; folded in via fused vector add
            nd_sb = [pmed.tile([128, 1024], F16, tag="ndsb", bufs=4, name=f"ndsb{k}")
                     for k in range(4)]
            for k in range(4):
                nc.sync.dma_start(
                    out=nd_sb[k][:].rearrange("p (r c) -> p r c", r=NCORE),
                    in_=agnd_out[:].rearrange("(r q p) c -> q p r c",
                                              r=NCORE, q=4)[k])
            cross = [pbig.tile([128, R], F16, tag="bigA", bufs=8, name=f"cross{k}")
                     for k in range(4)]
            for j in range(4):
                sl = slice(512 * j, 512 * (j + 1))
                om_j = []
                for k in range(4):
                    ps = pp.tile([128, 512], F32, tag="ps_mm", bufs=2, space="PSUM")
                    for ki in range(4):
                        nc.tensor.matmul(ps[:], lhsT=womic[ki][:, 128 * k:128 * (k + 1)],
                                         rhs=xt[ki][:, sl], start=(ki == 0), stop=(ki == 3))
                    a = pmed.tile([128, 512], F16, tag="omj", bufs=4)
                    bt = bpf[:, 12 + k:13 + k]
                    tom = pmed.tile([128, 512], F32, tag="tmpom", bufs=2)
                    nc.vector.tensor_scalar(out=tom[:], in0=ps[:], scalar1=bt,
                                            scalar2=SLOPE, op0=ALU.add, op1=ALU.mult)
                    nc.vector.tensor_scalar(out=a[:], in0=ps[:], scalar1=bt,
                                            scalar2=None, op0=ALU.add)
                    nc.vector.tensor_tensor(out=a[:], in0=a[:], in1=tom[:], op=ALU.max)
                    om_j.append(a)
                for k in range(4):
                    ps = pp.tile([128, 512], F32, tag="ps_mm", bufs=2, space="PSUM")
                    for ki in range(4):
                        nc.tensor.matmul(ps[:], lhsT=wfom[ki][:, 128 * k:128 * (k + 1)],
                                         rhs=om_j[ki][:], start=(ki == 0),
                                         stop=(ki == 3))
                    nc.vector.tensor_copy(out=cross[k][:, sl], in_=ps[:])
                    # + tiled cross_nd (fus_b asserted zero) on GpSimd so the
                    # vector/PSUM pipeline never waits on AG-nd
                    e0 = 512 * (j % 2)
                    nc.gpsimd.tensor_tensor(out=cross[k][:, sl],
                                            in0=cross[k][:, sl],
                                            in1=nd_sb[k][:, e0:e0 + 512], op=ALU.add)
            nc.sync.dma_start(out=cross[3][127:128, :], in_=x_t[511:512, :])
            if DEBUG:
                for k in range(4):
                    nc.sync.dma_start(out=dbg_cross[128 * k:128 * (k + 1), :],
                                      in_=cross[k][:])

            # ---- generic gather+scatter ----
            def _bounds(nch_t):
                b = []
                for t_id, nch in enumerate(nch_t):
                    for j in range(nch):
                        b.append((t_id, j == 0, j == nch - 1))
                return b

            def scatter(src_dram, idx_t, sel_d, sel_dt, nchunks, tile_bounds,
                        psum_tag, gbufs_n, sfx=""):
                out_psums = []
                ps = None
                src_ap = src_dram if isinstance(src_dram, bass.AP) else src_dram[:]
                cur = 0
                i = 0
                for w, s in enumerate(_wave_sizes(nchunks)):
                    g = pg.tile([128, WAVE, 512], sel_dt, tag="gath" + sfx,
                                bufs=gbufs_n)
                    nc.gpsimd.dma_gather(
                        g[:, :s, :], src_ap, idx_t[:, 8 * cur:8 * (cur + s)],
                        s * 128, s * 128, 512,
                        single_packet=True, queue_num=w % 4)
                    sw = pg.tile([128, WAVE, 128], sel_dt, tag="selw" + sfx,
                                 bufs=gbufs_n)
                    nc.sync.dma_start(
                        out=sw[:, :s, :].rearrange("p a d -> p (a d)"),
                        in_=sel_d[:, 128 * cur:128 * (cur + s)])
                    for slot in range(s):
                        t_id, first, last = tile_bounds[i]
                        i += 1
                        if first:
                            ps = pp.tile([128, 512], F32, tag=psum_tag, bufs=2,
                                         space="PSUM")
                        nc.tensor.matmul(ps[:], lhsT=sw[:, slot, :],
                                         rhs=g[:, slot, :],
                                         start=first, stop=last)
                        if last:
                            out_psums.append((t_id, ps))
                    cur += s
                return out_psums

            # ---- m1 (row-major fp16) + AG1 ----
            wienc = [pw.tile([128, 512], F16, tag="wres", bufs=12, name=f"wienc{k}")
                     for k in range(4)]
            for k in range(4):
                nc.sync.dma_start(out=wienc[k][:], in_=ienc_W[128 * k:128 * (k + 1), :])
            for t in range(NT):
                tsl = slice(128 * t, 128 * (t + 1))
                ps = pp.tile([128, 512], F32, tag="ps_mm", bufs=2, space="PSUM")
                for ki in range(4):
                    nc.tensor.matmul(ps[:], lhsT=cross[ki][:, tsl], rhs=wienc[ki][:],
                                     start=(ki == 0), stop=(ki == 3))
                h = pmed.tile([128, 512], F16, tag="m1h", bufs=3)
                nc.vector.tensor_copy(out=h[:], in_=ps[:])
                nc.sync.dma_start(out=ag1_in[tsl, :], in_=h[:])
                h8 = pmed.tile([128, 512], F8, tag="m1h8", bufs=3)
                nc.scalar.activation(h8[:], ps[:], ACTF.Copy)
                nc.sync.dma_start(
                    out=ag8_in[32 * t:32 * (t + 1), :].rearrange(
                        "a (b f) -> (a b) f", b=4),
                    in_=h8[:])
            nc.gpsimd.collective_compute(
                "AllGather", ALU.bypass, replica_groups=RG,
                ins=[ag8_in[:]], outs=[ag8_out[:]])
            # ---- zpre (in place on xt; pre_b asserted zero). The pre_t
            # streams are prefetched in full so the scheduler sees zpre as
            # ready work for the AG1 window (before the gather-fed matmuls).
            wpre = [pw.tile([128, 512], F16, tag="wres", bufs=12, name=f"wpre{k}")
                    for k in range(4)]
            for k in range(4):
                nc.sync.dma_start(out=wpre[k][:], in_=pre_W[128 * k:128 * (k + 1), :])
            pre_all = {}
            for j in range(4):
                for ki in range(4):
                    s = pmed.tile([128, 512], F16, tag="prestream", bufs=16)
                    nc.sync.dma_start(
                        out=s[:],
                        in_=pre_t_d[128 * ki:128 * (ki + 1),
                                    slice(512 * j, 512 * (j + 1))])
                    pre_all[j, ki] = s
            for j in range(4):
                sl = slice(512 * j, 512 * (j + 1))
                pre_j = [pre_all[j, ki] for ki in range(4)]
                for k in range(4):
                    ps = pp.tile([128, 512], F32, tag="ps_mm", bufs=2, space="PSUM")
                    for ki in range(4):
                        nc.tensor.matmul(ps[:], lhsT=wpre[ki][:, 128 * k:128 * (k + 1)],
                                         rhs=pre_j[ki][:], start=(ki == 0),
                                         stop=(ki == 3))
                    nc.vector.tensor_tensor(out=xt[k][:, sl], in0=xt[k][:, sl],
                                            in1=ps[:], op=ALU.add)

            # ---- gconv1 local-src edges: gathered from ag1_in DURING AG1 ----
            uacc = [pmed.tile([128, 512], F16, tag="uacc", bufs=NT, name=f"uacc{t}")
                    for t in range(NT)]
            seg1l = scatter(ag1_in, idxs["1l"], sel1l_d, F16, C1L,
                            _bounds(nch1l_t), "ps_seg", 2)
            for t_id, ps in seg1l:
                nc.vector.tensor_copy(out=uacc[t_id][:], in_=ps[:])

            # ---- enc weights (used by fused z @ enc_W after the scatter) ----
            wenc = [pw.tile([128, 512], F16, tag="wres", bufs=12, name=f"wenc{k}")
                    for k in range(4)]
            for k in range(4):
                nc.sync.dma_start(out=wenc[k][:], in_=enc_W[128 * k:128 * (k + 1), :])

            ag8_rows = ag8_out[:].rearrange("a (b f) -> (a b) f", b=4)
            seg1r = scatter(ag8_rows, idxs["1r"], sel1r_d, F8, C1R,
                            _bounds(nch1r_t), "ps_seg", 4, sfx="8")

            # ---- z^T = zpre^T + lrelu(gconv1)^T, accumulated in place in xt ----
            for q in range(4):
                pst = [pp.tile([128, 512], F32, tag="ps_ut", bufs=4, space="PSUM",
                               name=f"pst{q}_{k_}") for k_ in range(4)]
                for tt in range(4):
                    t_id, ps = seg1r[4 * q + tt]
                    useg = pmed.tile([128, 512], F32, tag="useg", bufs=3)
                    nc.vector.tensor_tensor(out=useg[:], in0=ps[:],
                                            in1=uacc[t_id][:], op=ALU.add)
                    for k in range(4):
                        nc.tensor.transpose(
                            out=pst[k][:, 128 * tt:128 * (tt + 1)],
                            in_=useg[:, 128 * k:128 * (k + 1)], identity=ident[:])
                for k in range(4):
                    sl = slice(512 * q, 512 * (q + 1))
                    tmp = pmed.tile([128, 512], F32, tag="lrtmp", bufs=2)
                    nc.vector.tensor_scalar(
                        out=tmp[:], in0=pst[k][:], scalar1=bpf[:, 16 + k:17 + k],
                        scalar2=SLOPE, op0=ALU.add, op1=ALU.mult)
                    u16 = pmed.tile([128, 512], F16, tag="u16", bufs=2)
                    nc.vector.tensor_scalar(
                        out=u16[:], in0=pst[k][:], scalar1=bpf[:, 16 + k:17 + k],
                        scalar2=None, op0=ALU.add)
                    nc.vector.tensor_tensor(
                        out=u16[:], in0=u16[:], in1=tmp[:], op=ALU.max)
                    nc.vector.tensor_tensor(
                        out=xt[k][:, sl], in0=xt[k][:, sl], in1=u16[:], op=ALU.add)

            # ---- m2 = z @ enc_W (fp16, stays local; stored in two halves so
            # gconv2's first gather waves overlap the second half) ----
            for t in range(NT):
                tsl = slice(128 * t, 128 * (t + 1))
                ps = pp.tile([128, 512], F32, tag="ps_mm", bufs=2, space="PSUM")
                for ki in range(4):
                    nc.tensor.matmul(ps[:], lhsT=xt[ki][:, tsl], rhs=wenc[ki][:],
                                     start=(ki == 0), stop=(ki == 3))
                h = pmed.tile([128, 512], F16, tag="m2h", bufs=3)
                nc.vector.tensor_copy(out=h[:], in_=ps[:])
                if t < 12:
                    nc.sync.dma_start(out=m2_a[tsl, :], in_=h[:])
                else:
                    nc.sync.dma_start(
                        out=m2_b[128 * (t - 12):128 * (t - 11), :], in_=h[:])

            # ---- gconv2: source-side partials over 1024 slots + ReduceScatter ----
            acc2 = [pmed.tile([128, 512], F16, tag="acc2", bufs=8, name=f"acc2{t}")
                    for t in range(8)]
            seg2a = scatter(m2_a, idxs["2a"], sel2a_d, F16, C2A,
                            _bounds(nch2a_t), "ps_seg", 2)
            for t_id, ps in seg2a:
                nc.vector.tensor_copy(out=acc2[t_id][:], in_=ps[:])
            seg2b = scatter(m2_b, idxs["2b"], sel2b_d, F16, C2B,
                            _bounds(nch2b_t), "ps_seg", 2)
            for t_id, ps in seg2b:
                pc = pmed.tile([128, 512], F16, tag="m2h", bufs=3)
                nc.vector.tensor_tensor(out=pc[:], in0=ps[:], in1=acc2[t_id][:],
                                        op=ALU.add)
                nc.sync.dma_start(
                    out=rs_in[32 * t_id:32 * (t_id + 1), :].rearrange(
                        "a (b f) -> (a b) f", b=4),
                    in_=pc[:])
            nc.gpsimd.collective_compute(
                "ReduceScatter", ALU.add, replica_groups=RG,
                ins=[rs_in[:]], outs=[rs_out[:]])

            # ---- zk^T + readout ----
            zkr16 = pmed.tile([128, 512], F16, tag="m2h", bufs=3)
            nc.sync.dma_start(
                out=zkr16[:],
                in_=rs_out[:].rearrange("a (b f) -> (a b) f", b=4))
            zkr = pmed.tile([128, 512], F32, tag="useg", bufs=3)
            nc.vector.tensor_copy(out=zkr[:], in_=zkr16[:])
            if DEBUG:
                nc.sync.dma_start(out=dbg_zk[:], in_=zkr[:])
            ps_zt = pp.tile([128, 512], F32, tag="ps_ut", bufs=4, space="PSUM")
            for k in range(4):
                nc.tensor.transpose(out=ps_zt[:, 128 * k:128 * (k + 1)],
                                    in_=zkr[:, 128 * k:128 * (k + 1)], identity=ident[:])
            zkt = pmed.tile([128, 512], F32, tag="zkt", bufs=1)
            for k in range(4):
                sl = slice(128 * k, 128 * (k + 1))
                tmp = pmed.tile([128, 128], F32, tag="lrtmp2", bufs=2)
                nc.vector.tensor_scalar(
                    out=tmp[:], in0=ps_zt[:, sl], scalar1=bpf[:, 20 + k:21 + k],
                    scalar2=SLOPE, op0=ALU.add, op1=ALU.mult)
                nc.vector.tensor_scalar(
                    out=zkt[:, sl], in0=ps_zt[:, sl], scalar1=bpf[:, 20 + k:21 + k],
                    scalar2=None, op0=ALU.add)
                nc.vector.tensor_tensor(
                    out=zkt[:, sl], in0=zkt[:, sl], in1=tmp[:], op=ALU.max)

            wg1 = [pw.tile([128, 512], F32, tag="wres32", bufs=4, name=f"wg1{k}")
                   for k in range(4)]
            for k in range(4):
                nc.sync.dma_start(out=wg1[k][:], in_=gate_W1[128 * k:128 * (k + 1), :])
            w2r = psc.tile([128, 8], F32, tag="w2r", bufs=1)
            nc.sync.dma_start(out=w2r[:], in_=gw2reg[:])
            s1t = pmed.tile([128, 512], F32, tag="s1t", bufs=1)
            for ko_ in range(4):
                ps = pp.tile([128, 512], F32, tag="ps_mm", bufs=2, space="PSUM")
                for ki in range(4):
                    nc.tensor.matmul(ps[:, :128],
                                     lhsT=wg1[ki][:, 128 * ko_:128 * (ko_ + 1)],
                                     rhs=zkt[:, 128 * ki:128 * (ki + 1)],
                                     start=(ki == 0), stop=False)
                nc.tensor.matmul(ps[:, :128],
                                 lhsT=brow_g[:, 128 * ko_:128 * (ko_ + 1)],
                                 rhs=ones[:, :128], start=False, stop=True)
                nc.scalar.activation(s1t[:, 128 * ko_:128 * (ko_ + 1)], ps[:, :128],
                                     ACTF.Tanh)
            ps_sc = pp.tile([128, 512], F32, tag="ps_mm", bufs=2, space="PSUM")
            for ki in range(4):
                nc.tensor.matmul(ps_sc[:1, :128], lhsT=w2r[:, 2 * ki:2 * ki + 1],
                                 rhs=s1t[:, 128 * ki:128 * (ki + 1)],
                                 start=(ki == 0), stop=(ki == 3))
            ps_tr = pp.tile([128, 512], F32, tag="ps_seg", bufs=2, space="PSUM")
            for ki in range(4):
                nc.tensor.matmul(ps_tr[:1, :128], lhsT=w2r[:, 2 * ki + 1:2 * ki + 2],
                                 rhs=zkt[:, 128 * ki:128 * (ki + 1)],
                                 start=(ki == 0), stop=(ki == 3))
            erow = psc.tile([1, 128], F32, tag="erow", bufs=1)
            nc.scalar.activation(erow[:], ps_sc[:1, :128], ACTF.Exp,
                                 bias=bpf[:1, 24:25])
            etrow = psc.tile([1, 128], F32, tag="etrow", bufs=1)
            nc.vector.tensor_tensor(out=etrow[:], in0=erow[:], in1=ps_tr[:1, :128],
                                    op=ALU.mult)
            sums = psc.tile([1, 4], F32, tag="sums", bufs=1)
            nc.vector.tensor_reduce(out=sums[:, 0:2],
                                    in_=etrow[:].rearrange("p (g x) -> p g x", g=2),
                                    axis=AX, op=ALU.add)
            nc.vector.tensor_reduce(out=sums[:, 2:4],
                                    in_=erow[:].rearrange("p (g x) -> p g x", g=2),
                                    axis=AX, op=ALU.add)
            res = psc.tile([1, 4], F32, tag="res", bufs=1)
            nc.vector.reciprocal(out=res[:, 2:4], in_=sums[:, 2:4])
            nc.vector.tensor_tensor(out=res[:, 0:2], in0=sums[:, 0:2],
                                    in1=res[:, 2:4], op=ALU.mult)
            nc.vector.tensor_scalar(out=res[:, 0:2], in0=res[:, 0:2],
                                    scalar1=bpf[:1, 25:26], scalar2=None, op0=ALU.add)
            nc.sync.dma_start(out=out_d[:], in_=res[:, 0:2])

    nc.compile()
    return nc


def _ensure_ntff_hook():
    """Inject antenv.axon_hooks (absent in this image) so trace=True works."""
    import sys, types
    try:
        from antenv.axon_hooks import get_axon_ntff_profile_hook  # noqa
        return
    except ImportError:
        pass
    import antenv
    mod = types.ModuleType("antenv.axon_hooks")
    _state = {"hook": None}
    mod.set_axon_ntff_profile_hook = lambda h: _state.__setitem__("hook", h)
    mod.get_axon_ntff_profile_hook = lambda: _state["hook"]
    sys.modules["antenv.axon_hooks"] = mod
    antenv.axon_hooks = mod
    from trn_agent_boot.trn_boot import _ntff_profile_via_ctypes
    mod.set_axon_ntff_profile_hook(
        _ntff_profile_via_ctypes("/opt/axon/libaxon_pjrt.so"))


# ---------------------------------------------------------------------------
# host wrapper
# ---------------------------------------------------------------------------

def kernel(**inputs):
    f32 = lambda k: np.asarray(inputs[k], np.float32)
    x = f32("x"); pre_x = f32("pre_x")
    edge_index = np.asarray(inputs["edge_index"], np.int64)
    internal_edge_index = np.asarray(inputs["internal_edge_index"], np.int64)
    name_emb = f32("name_embeddings"); desc_emb = f32("desc_embeddings")
    ko_mask = np.asarray(inputs["ko_mask"], np.int64)
    bkm = np.asarray(inputs["batch_ko_masks"], np.int64)
    name_W = f32("name_W"); name_b = f32("name_b")
    desc_W = f32("desc_W"); desc_b = f32("desc_b")
    omic_W = f32("omic_W"); omic_b = f32("omic_b")
    fus_W = f32("fus_W"); fus_b = f32("fus_b")
    pre_W = f32("pre_W"); pre_b = f32("pre_b")
    ienc_W = f32("ienc_W"); ienc_b = f32("ienc_b")
    enc_W = f32("enc_W"); enc_b = f32("enc_b")
    gate_W1 = f32("gate_W1"); gate_b1 = f32("gate_b1")
    gate_W2 = f32("gate_W2"); gate_b2 = f32("gate_b2")
    reg_W = f32("reg_W"); reg_b = f32("reg_b")

    assert not fus_b.any() and not pre_b.any(), \
        "nonzero fus_b/pre_b not supported by this build"

    ko_feat = np.zeros(N, np.float32)
    ko_feat[ko_mask] = 1.0

    # ---- gconv2: source-sharded edges into the 1024 global KO slots ----
    slot_row = (bkm + np.arange(B)[:, None] * NE).reshape(-1)   # [1024]
    row2slots = {}
    for s_, r_ in enumerate(slot_row):
        row2slots.setdefault(int(r_), []).append(s_)
    def _pad_last(nch_t):
        nch_t[-1] += (-int(nch_t.sum())) % WAVE
        return tuple(int(v) for v in nch_t)

    s2_all, d2_all = edge_index[0], edge_index[1]
    m2mask = np.isin(d2_all, slot_row)
    per_core_2a = []   # sources in local rows [0, R/2)
    per_core_2b = []   # sources in local rows [R/2, R)
    needed = []        # per-core local rows whose z/m2 is actually consumed
    nch2a_t = np.ones(8, np.int64)
    nch2b_t = np.ones(8, np.int64)
    for c in range(NCORE):
        lo, hi = R * c, R * (c + 1)
        ss, ds = [], []
        for r_, sl_ in row2slots.items():
            if lo <= r_ < hi:
                for s_ in sl_:
                    ss.append(r_ - lo); ds.append(s_)
        mm = m2mask & (s2_all >= lo) & (s2_all < hi)
        for u, v in zip(s2_all[mm], d2_all[mm]):
            for s_ in row2slots[int(v)]:
                ss.append(int(u) - lo); ds.append(s_)
        src = np.array(ss, np.int64); dstl = np.array(ds, np.int64)
        nd = np.zeros(R, bool)
        nd[src] = True
        needed.append(nd)
        ha = src < 12 * 128
        per_core_2a.append((src[ha], dstl[ha]))
        per_core_2b.append((src[~ha] - 12 * 128, dstl[~ha]))
        nch2a_t = np.maximum(nch2a_t, -(-np.bincount(dstl[ha] >> 7, minlength=8) // 128))
        nch2b_t = np.maximum(nch2b_t, -(-np.bincount(dstl[~ha] >> 7, minlength=8) // 128))
    nch2a_t = _pad_last(nch2a_t)
    nch2b_t = _pad_last(nch2b_t)

    # ---- gconv1 edges (dst-sharded; self term added from local m1h).
    # Edges whose dst row never feeds gconv2 (not a slot row, not a source of
    # a slot edge) are dropped: their z rows are never read. Edges with a
    # LOCAL source are gathered from ag1_in during AG1. ----
    s1_all, d1_all = internal_edge_index[0], internal_edge_index[1]
    per_core_1l = []
    per_core_1r = []
    nch1l_t = np.ones(NT, np.int64)
    nch1r_t = np.ones(NT, np.int64)
    for c in range(NCORE):
        lo, hi = R * c, R * (c + 1)
        m = (d1_all >= lo) & (d1_all < hi)
        s1 = s1_all[m]; d1l = d1_all[m] - lo
        keep = needed[c][d1l]
        s1 = s1[keep]; d1l = d1l[keep]
        isloc = (s1 >= lo) & (s1 < hi)
        # the gconv self term rides the local pass as synthetic (r -> r) edges
        selfr = np.nonzero(needed[c])[0]
        ls = np.concatenate([s1[isloc] - lo, selfr])
        ld = np.concatenate([d1l[isloc], selfr])
        per_core_1l.append((ls, ld))
        per_core_1r.append((s1[~isloc], d1l[~isloc]))
        nch1l_t = np.maximum(
            nch1l_t, -(-np.bincount(ld >> 7, minlength=NT) // 128))
        nch1r_t = np.maximum(
            nch1r_t, -(-np.bincount(d1l[~isloc] >> 7, minlength=NT) // 128))
    nch1l_t = _pad_last(nch1l_t)
    nch1r_t = _pad_last(nch1r_t)

    nc = _build(nch1l_t, nch1r_t, nch2a_t, nch2b_t)

    import ml_dtypes
    f16 = np.float16
    f8 = ml_dtypes.float8_e4m3
    omic_Wp = _pad_w(omic_W, 512, 512)
    fus_ndp = _pad_w(fus_W[:2 * TX], 2 * TX, 512)
    fus_omp = _pad_w(fus_W[2 * TX:], 512, 512)
    # [p, mo, ki, m] = W[ki*128+p, mo*128+m] so wstrip loads are contiguous
    name_Wr = np.ascontiguousarray(
        name_W.reshape(6, 128, 6, 128).transpose(1, 2, 0, 3).reshape(128, 6 * TX))
    desc_Wr = np.ascontiguousarray(
        desc_W.reshape(6, 128, 6, 128).transpose(1, 2, 0, 3).reshape(128, 6 * TX))
    fus_ndr = np.ascontiguousarray(
        fus_ndp.reshape(12, 128, 4, 128).transpose(1, 2, 0, 3).reshape(128, 6144))
    bias_pf = np.zeros((128, 26), np.float32)
    bias_pf[:, 0:6] = name_b.reshape(6, 128).T
    bias_pf[:, 6:12] = desc_b.reshape(6, 128).T
    bias_pf[:, 12:16] = _pad_w(omic_b[:, None], 512, 1).reshape(4, 128).T
    bias_pf[:, 16:20] = ienc_b.reshape(4, 128).T
    bias_pf[:, 20:24] = enc_b.reshape(4, 128).T
    bias_pf[:, 24] = float(gate_b2.reshape(-1)[0])
    bias_pf[:, 25] = float(reg_b.reshape(-1)[0])
    bias_rows = np.zeros((96, 512), np.float32)
    bias_rows[64, :] = gate_b1
    gw2 = np.concatenate([gate_W2, reg_W], axis=1).astype(np.float32)
    gw2 = np.ascontiguousarray(
        gw2.reshape(4, 128, 2).transpose(1, 0, 2).reshape(128, 8))

    shared = dict(
        name_W=name_Wr.astype(f16), desc_W=desc_Wr.astype(f16),
        omic_W=omic_Wp.astype(f16), fus_nd=fus_ndr.astype(f16),
        fus_om=fus_omp.astype(f16), ienc_W=ienc_W.astype(f16),
        pre_W=pre_W.astype(f16), enc_W=enc_W.astype(f16),
        gate_W1=gate_W1, gw2reg=gw2, bias_pf=bias_pf, bias_rows=bias_rows,
    )

    in_maps = []
    for c in range(NCORE):
        lo, hi = R * c, R * (c + 1)
        x_t = np.concatenate([x[lo:hi].T, ko_feat[None, lo:hi]], 0)
        pre_t = np.concatenate([pre_x[lo:hi].T, ko_feat[None, lo:hi]], 0)
        ndemb = np.concatenate(
            [name_emb[128 * c:128 * (c + 1)].T, desc_emb[128 * c:128 * (c + 1)].T], 0)
        ndemb = ndemb.reshape(12, 128, 128).transpose(1, 0, 2).reshape(128, 12 * 128)
        i1l, dv1l = _chunk_edges_per_tile(*per_core_1l[c], nch1l_t)
        i1r, dv1r = _chunk_edges_per_tile(*per_core_1r[c], nch1r_t)
        i2a, dv2a = _chunk_edges_per_tile(*per_core_2a[c], nch2a_t)
        i2b, dv2b = _chunk_edges_per_tile(*per_core_2b[c], nch2b_t)
        in_maps.append(dict(
            x_t=np.ascontiguousarray(x_t).astype(f16),
            pre_t=np.ascontiguousarray(pre_t).astype(f16),
            ndemb=np.ascontiguousarray(ndemb).astype(f16),
            idx1l=_wrap_idx_waves(i1l), sel1l=_sel_from_dstv(dv1l, f16),
            idx1r=_wrap_idx_waves(i1r), sel1r=_sel_from_dstv(dv1r, f8),
            idx2a=_wrap_idx_waves(i2a), sel2a=_sel_from_dstv(dv2a, f16),
            idx2b=_wrap_idx_waves(i2b), sel2b=_sel_from_dstv(dv2b, f16),
            **shared,
        ))

    if TRACE:
        _ensure_ntff_hook()
    res = run_bass_kernel_spmd(nc, in_maps, core_ids=list(range(NCORE)),
                               trace=TRACE, **(TRACE_KW or {}))
    kernel._last = res
    out = np.zeros(B, np.float32)
    for c in range(NCORE):
        out[2 * c:2 * c + 2] = res.results[c]["out"][0]
    return out

